# revision 25
# baseline (speedup 1.0000x reference)
# Multi-head causal attention (B=4, T=2048, C=1024, H=16, DH=64) on 8 trn2 cores.
# Sharding: core c -> batch b=c//2 (data parallel), head group g=c%2 (8 heads,
# tensor parallel). Wo is row-sharded over the head dim; the cross-group
# reduction (+bias) happens on the host during the gather.
import numpy as np
import ml_dtypes

B, T, C = 4, 2048, 1024
H, DH = 16, 64
N_CORES = 8
HPC = 8            # heads per core
NPAIR = HPC // 2   # head pairs per core (PE row-group packing)
KC = C // 128      # contraction chunks for the input projections
NT = T // 128      # 128-row tiles of T
NS = T // 512      # 512-col stripes of T
HD = HPC * DH      # 512: per-core concat-head width
SCALE = DH ** -0.5

_CACHE = {}


def _patch_framework(tile_mod, mybir):
    """This toolchain's walrus build accepts at most ONE semaphore wait per
    instruction. Tile freely assigns several, and its end-of-kernel drain
    collects one per outstanding proc. Patch the drain to pre-consume waits
    one NOP at a time; a post-pass splits any remaining multi-wait
    instruction into single-wait NOPs + the instruction."""
    if getattr(tile_mod.TileContext, "_onewait_patched", False):
        return
    from concourse.vector_clock import ScopedClock, VectorClock

    _orig = tile_mod.TileContext._drain_and_barrier

    def _patched(self, tick_clock, wait_clock):
        nc = self.nc
        gc = tick_clock.global_clock
        for proc in range(len(gc)):
            t = gc[proc]
            if t > 0:
                vec = [0] * len(gc)
                vec[proc] = t
                nop_inst = nc.sync.nop()
                wait_clock.add_sem_waits(
                    nop_inst.ins, ScopedClock({None: VectorClock(vec)})
                )
        _orig(self, tick_clock, wait_clock)

    tile_mod.TileContext._drain_and_barrier = _patched
    tile_mod.TileContext._onewait_patched = True


def _split_multi_waits(nc, mybir):
    cnt = 0
    for f in nc.m.functions:
        for bb in f.blocks:
            insts = list(bb.instructions)
            out = []
            changed = False
            for inst in insts:
                si = getattr(inst, "sync_info", None)
                if si is not None and si.on_wait and len(si.on_wait) > 1:
                    waits = list(si.on_wait)
                    for w in waits[:-1]:
                        cnt += 1
                        nop = mybir.InstNoOp(
                            name=f"wsplit_{cnt}_{inst.name}", ins=[], outs=[]
                        )
                        nop.engine = inst.engine
                        nop.sync_info = mybir.SyncInfo(on_wait=[w], on_update=[])
                        out.append(nop)
                    inst.sync_info = mybir.SyncInfo(
                        on_wait=[waits[-1]], on_update=list(si.on_update)
                    )
                    changed = True
                out.append(inst)
            if changed:
                bb.instructions = out


def build_bass():
    import concourse.bass as bass
    import concourse.mybir as mybir
    import concourse.tile as tile

    _patch_framework(tile, mybir)

    f32 = mybir.dt.float32
    f32r = mybir.dt.float32r
    bf16 = mybir.dt.bfloat16
    Exp = mybir.ActivationFunctionType.Exp

    nc = bass.Bass("TRN2", target_bir_lowering=False, debug=False,
                   enable_asserts=False)

    qT = nc.dram_tensor("qT", [C, T], bf16, kind="ExternalInput").ap()
    kT = nc.dram_tensor("kT", [C, T], bf16, kind="ExternalInput").ap()
    vT = nc.dram_tensor("vT", [C, T], bf16, kind="ExternalInput").ap()
    wq = nc.dram_tensor("wq", [C, HD], bf16, kind="ExternalInput").ap()
    wk = nc.dram_tensor("wk", [C, HD], bf16, kind="ExternalInput").ap()
    wv = nc.dram_tensor("wv", [C, HD], bf16, kind="ExternalInput").ap()
    wo = nc.dram_tensor("wo", [HD, C], f32, kind="ExternalInput").ap()
    out = nc.dram_tensor("out", [T, C], f32, kind="ExternalOutput").ap()

    with tile.TileContext(nc) as tc:
        with (
            tc.tile_pool(name="persist", bufs=1) as persist,
            tc.tile_pool(name="stage", bufs=3) as stage,
            tc.tile_pool(name="work", bufs=2) as work,
            tc.tile_pool(name="pt_pool", bufs=6) as pt_pool,
            tc.tile_pool(name="ps_proj", bufs=2, space="PSUM") as ps_proj,
            tc.tile_pool(name="ps_st", bufs=2, space="PSUM") as ps_st,
            tc.tile_pool(name="ps_ot", bufs=1, space="PSUM") as ps_ot,
        ):
            # ---- weights (loaded lazily, per phase; only wq gates start) ----
            wq_t = persist.tile([128, KC, HD], bf16)
            wk_t = persist.tile([128, KC, HD], bf16)
            wv_t = persist.tile([128, KC, HD], bf16)
            nc.sync.dma_start(wq_t[:], wq.rearrange("(o p) n -> p o n", p=128))

            # ---- projections ----
            # qhT/khT: [128 (pair-local head dim), NPAIR, T] f32r. Partition
            # p in pair m: head 2m for p<64, head 2m+1 for p>=64.
            qh_t = persist.tile([128, NPAIR, T], f32r)
            kh_t = persist.tile([128, NPAIR, T], f32r)
            # vh: [128 (T within tile), NT, HPC, 65] f32r; col 64 is ones
            # (softmax-denominator trick), cols 0..63 hold vh.
            # memset everything to 1; the projection copybacks overwrite
            # cols 0..63, leaving col 64 as the ones column.
            vh_t = persist.tile([128, NT, HPC, 65], bf16)
            nc.gpsimd.memset(vh_t[:, :, :, 64:65], 1.0)

            # causal mask tile: mask[p, c] = 1 if (c - 384) >= p else 0.
            # Diagonal block (i, j) with d = 128*i - 512*j uses the slice
            # mask[:, 384 - d : 896 - d]  (keep iff f - p >= d).
            mask_t = persist.tile([128, 896], bf16)
            nc.gpsimd.memset(mask_t[:], 1.0)
            nc.gpsimd.affine_select(
                mask_t[:], mask_t[:],
                compare_op=mybir.AluOpType.is_ge, fill=0.0,
                base=-384, pattern=[[1, 896]], channel_multiplier=-1,
            )

            # ---- pipelined: project half of T, then run its two
            # attention stripes while the other half projects ----
            def stage_quarter(src_ap, j):
                x_t = stage.tile([128, KC, 512], bf16, tag="x_stage")
                nc.gpsimd.dma_start(
                    x_t[:],
                    src_ap.rearrange("(o p) t -> p o t", p=128)[
                        :, :, j * 512:(j + 1) * 512],
                )
                return x_t

            def proj_qk(src_ap, w_t, dst, j):
                x_t = stage_quarter(src_ap, j)
                for m in range(NPAIR):
                    ps = ps_proj.tile([128, 512], f32, tag="proj")
                    for k in range(KC):
                        nc.tensor.matmul(
                            ps[:],
                            w_t[:, k, m * 128:(m + 1) * 128],
                            x_t[:, k, :],
                            start=(k == 0), stop=(k == KC - 1),
                        )
                    nc.scalar.copy(dst[:, m, j * 512:(j + 1) * 512], ps[:])

            def proj_v(j):
                x_t = stage_quarter(vT, j)
                for tt in range(4 * j, 4 * j + 4):
                    ps = ps_proj.tile([128, 512], f32, tag="proj")
                    off = tt * 128 - j * 512
                    for k in range(KC):
                        nc.tensor.matmul(
                            ps[:],
                            x_t[:, k, off:off + 128],
                            wv_t[:, k, :],
                            start=(k == 0), stop=(k == KC - 1),
                        )
                    nc.scalar.copy(
                        vh_t[:, tt, :, 0:64],
                        ps[:].rearrange("p (h d) -> p h d", h=HPC),
                    )

            def attention_stripe(j):
                nblk = 4 * j + 4
                for p in range(NPAIR):
                    ot0 = ps_ot.tile([128, 512], f32, tag="ot0")
                    ot1 = ps_ot.tile([128, 512], f32, tag="ot1")
                    for i in range(nblk):
                        st = ps_st.tile([128, 1024], f32, tag="st")
                        # S^T block for both heads (row-group packed)
                        nc.tensor.matmul(
                            st[:, 0:512],
                            kh_t[0:64, p, i * 128:(i + 1) * 128],
                            qh_t[0:64, p, j * 512:(j + 1) * 512],
                            start=True, stop=True,
                        )
                        nc.tensor.matmul(
                            st[:, 512:1024],
                            kh_t[64:128, p, i * 128:(i + 1) * 128],
                            qh_t[64:128, p, j * 512:(j + 1) * 512],
                            start=True, stop=True,
                        )
                        pt = pt_pool.tile([128, 1024], bf16, tag="pt")
                        nc.scalar.activation(pt[:], st[:], Exp, scale=SCALE)
                        if i >= 4 * j:  # diagonal block: zero s > t entries
                            d = i * 128 - j * 512
                            ms = mask_t[:, None, 384 - d:896 - d]
                            nc.vector.tensor_tensor(
                                pt[:].rearrange("p (h f) -> p h f", h=2),
                                pt[:].rearrange("p (h f) -> p h f", h=2),
                                ms.to_broadcast((128, 2, 512)),
                                mybir.AluOpType.mult,
                            )
                        # PV: accumulate [O^T; l] over key blocks
                        nc.tensor.matmul(
                            ot0[0:65, :],
                            vh_t[:, i, 2 * p, 0:65],
                            pt[:, 0:512],
                            start=(i == 0), stop=(i == nblk - 1),
                        )
                        nc.tensor.matmul(
                            ot1[0:65, :],
                            vh_t[:, i, 2 * p + 1, 0:65],
                            pt[:, 512:1024],
                            start=(i == 0), stop=(i == nblk - 1),
                        )
                    # epilogue: divide O^T rows by l. Copy O^T out of PSUM
                    # right away so the ot banks turn around fast; head 1
                    # additionally needs a partition shift 0..64 -> 64..128
                    # (small SBUF->SBUF DMA; lanes are partition-locked).
                    rc0 = work.tile([128, 512], f32, tag="recip")
                    rc1 = work.tile([128, 512], f32, tag="recip")
                    nc.vector.reciprocal(rc0[64:65, :], ot0[64:65, :])
                    nc.vector.reciprocal(rc1[64:65, :], ot1[64:65, :])
                    tmp0 = work.tile([128, 512], f32, tag="tmp0")
                    tmp1 = work.tile([128, 512], f32, tag="tmp1")
                    nc.vector.tensor_copy(tmp0[0:64, :], ot0[0:64, :])
                    nc.vector.tensor_copy(tmp1[0:64, :], ot1[0:64, :])
                    bc = work.tile([128, 512], f32, tag="bcast")
                    nc.sync.dma_start(
                        bc[0:64, :],
                        rc0[64:65, None, :].to_broadcast((1, 64, 512)),
                    )
                    nc.sync.dma_start(
                        bc[64:128, :],
                        rc1[64:65, None, :].to_broadcast((1, 64, 512)),
                    )
                    ash = work.tile([128, 512], f32, tag="ash")
                    nc.sync.dma_start(ash[64:128, :], tmp1[0:64, :])
                    nc.vector.tensor_tensor(
                        qh_t[0:64, p, j * 512:(j + 1) * 512],
                        tmp0[0:64, :], bc[0:64, :], mybir.AluOpType.mult,
                    )
                    nc.vector.tensor_tensor(
                        qh_t[64:128, p, j * 512:(j + 1) * 512],
                        ash[64:128, :], bc[64:128, :], mybir.AluOpType.mult,
                    )
            def outproj_stripe(j):
                # output projection for a stripe (A^T aliased into qh_t)
                for tt in range(4 * j, 4 * j + 4):
                    for cc in range(2):
                        ps = ps_proj.tile([128, 512], f32, tag="proj")
                        for kc in range(NPAIR):
                            nc.tensor.matmul(
                                ps[:],
                                qh_t[:, kc, tt * 128:(tt + 1) * 128],
                                wo_t[:, kc, cc * 512:(cc + 1) * 512],
                                start=(kc == 0), stop=(kc == NPAIR - 1),
                            )
                        ob = work.tile([128, 512], f32, tag="ob")
                        nc.vector.tensor_copy(ob[:], ps[:])
                        nc.sync.dma_start(
                            out[tt * 128:(tt + 1) * 128, cc * 512:(cc + 1) * 512],
                            ob[:],
                        )

            for half in range(2):
                js = (2 * half, 2 * half + 1)
                for j in js:
                    proj_qk(qT, wq_t, qh_t, j)
                if half == 0:
                    nc.sync.dma_start(
                        wk_t[:], wk.rearrange("(o p) n -> p o n", p=128))
                for j in js:
                    proj_qk(kT, wk_t, kh_t, j)
                if half == 0:
                    nc.sync.dma_start(
                        wv_t[:], wv.rearrange("(o p) n -> p o n", p=128))
                for j in js:
                    proj_v(j)
                if half == 0:
                    wo_f = stage.tile([128, HD // 128, C], f32, tag="wo_stage", bufs=1)
                    nc.sync.dma_start(
                        wo_f[:], wo.rearrange("(o p) n -> p o n", p=128))
                    wo_t_ = persist.tile([128, HD // 128, C], f32r, name="wo_t")
                    nc.gpsimd.tensor_copy(wo_t_[:], wo_f[:])
                    wo_t = wo_t_
                else:
                    outproj_stripe(0)
                    outproj_stripe(1)
                for j in js:
                    attention_stripe(j)
            outproj_stripe(2)
            outproj_stripe(3)

    _split_multi_waits(nc, mybir)
    return nc


def shard_inputs(k, q, v, Wk, Wq, Wv, Wo, bo):
    bf = ml_dtypes.bfloat16
    qT = [np.ascontiguousarray(q[b].T).astype(bf) for b in range(B)]
    kT = [np.ascontiguousarray(k[b].T).astype(bf) for b in range(B)]
    vT = [np.ascontiguousarray(v[b].T).astype(bf) for b in range(B)]

    def wslice(W, g):
        return np.ascontiguousarray(
            W[g * HPC:(g + 1) * HPC].transpose(1, 0, 2).reshape(C, HD)
        ).astype(bf)

    wq = [wslice(Wq, g) for g in range(2)]
    wk = [wslice(Wk, g) for g in range(2)]
    wv = [wslice(Wv, g) for g in range(2)]
    wo = [np.ascontiguousarray(Wo[g * HD:(g + 1) * HD]).astype(np.float32)
          for g in range(2)]
    in_maps = []
    for c in range(N_CORES):
        b, g = divmod(c, 2)
        in_maps.append({
            "qT": qT[b], "kT": kT[b], "vT": vT[b],
            "wq": wq[g], "wk": wk[g], "wv": wv[g], "wo": wo[g],
        })
    return in_maps


def gather_outputs(results, bo):
    out = np.empty((B, T, C), dtype=np.float32)
    for b in range(B):
        out[b] = results[2 * b]["out"] + results[2 * b + 1]["out"] + bo
    return out


def kernel(k, q, v, Wk, Wq, Wv, Wo, bo):
    from concourse.bass_utils import run_bass_kernel_spmd

    k = np.asarray(k, dtype=np.float32)
    q = np.asarray(q, dtype=np.float32)
    v = np.asarray(v, dtype=np.float32)
    Wk = np.asarray(Wk, dtype=np.float32)
    Wq = np.asarray(Wq, dtype=np.float32)
    Wv = np.asarray(Wv, dtype=np.float32)
    Wo = np.asarray(Wo, dtype=np.float32)
    bo = np.asarray(bo, dtype=np.float32)

    if "nc" not in _CACHE:
        _CACHE["nc"] = build_bass()
    nc = _CACHE["nc"]
    in_maps = shard_inputs(k, q, v, Wk, Wq, Wv, Wo, bo)
    res = run_bass_kernel_spmd(nc, in_maps, core_ids=list(range(N_CORES)))
    return gather_outputs(res.results, bo)


# revision 28
# speedup vs baseline: 53.9423x; 53.9423x over previous
# Multi-head causal attention (B=4, T=2048, C=1024, H=16, DH=64) on 8 trn2 cores.
# Sharding: core c -> batch b=c//2 (data parallel), head group g=c%2 (8 heads,
# tensor parallel). Wo is row-sharded over the head dim; the cross-group
# reduction (+bias) happens on the host during the gather.
import numpy as np
import ml_dtypes

B, T, C = 4, 2048, 1024
H, DH = 16, 64
N_CORES = 8
HPC = 8            # heads per core
NPAIR = HPC // 2   # head pairs per core (PE row-group packing)
KC = C // 128      # contraction chunks for the input projections
NT = T // 128      # 128-row tiles of T
NS = T // 512      # 512-col stripes of T
HD = HPC * DH      # 512: per-core concat-head width
SCALE = DH ** -0.5

_CACHE = {}


def _patch_framework(tile_mod, mybir):
    """This toolchain's walrus build accepts at most ONE semaphore wait per
    instruction. Tile freely assigns several, and its end-of-kernel drain
    collects one per outstanding proc. Patch the drain to pre-consume waits
    one NOP at a time; a post-pass splits any remaining multi-wait
    instruction into single-wait NOPs + the instruction."""
    if getattr(tile_mod.TileContext, "_onewait_patched", False):
        return
    from concourse.vector_clock import ScopedClock, VectorClock

    _orig = tile_mod.TileContext._drain_and_barrier

    def _patched(self, tick_clock, wait_clock):
        nc = self.nc
        gc = tick_clock.global_clock
        for proc in range(len(gc)):
            t = gc[proc]
            if t > 0:
                vec = [0] * len(gc)
                vec[proc] = t
                nop_inst = nc.sync.nop()
                wait_clock.add_sem_waits(
                    nop_inst.ins, ScopedClock({None: VectorClock(vec)})
                )
        _orig(self, tick_clock, wait_clock)

    tile_mod.TileContext._drain_and_barrier = _patched
    tile_mod.TileContext._onewait_patched = True


def _split_multi_waits(nc, mybir):
    cnt = 0
    for f in nc.m.functions:
        for bb in f.blocks:
            insts = list(bb.instructions)
            out = []
            changed = False
            for inst in insts:
                si = getattr(inst, "sync_info", None)
                if si is not None and si.on_wait and len(si.on_wait) > 1:
                    waits = list(si.on_wait)
                    for w in waits[:-1]:
                        cnt += 1
                        nop = mybir.InstNoOp(
                            name=f"wsplit_{cnt}_{inst.name}", ins=[], outs=[]
                        )
                        nop.engine = inst.engine
                        nop.sync_info = mybir.SyncInfo(on_wait=[w], on_update=[])
                        out.append(nop)
                    inst.sync_info = mybir.SyncInfo(
                        on_wait=[waits[-1]], on_update=list(si.on_update)
                    )
                    changed = True
                out.append(inst)
            if changed:
                bb.instructions = out


def build_bass(reps=1):
    import concourse.bass as bass
    import concourse.mybir as mybir
    import concourse.tile as tile

    _patch_framework(tile, mybir)

    f32 = mybir.dt.float32
    f32r = mybir.dt.float32r
    bf16 = mybir.dt.bfloat16
    Exp = mybir.ActivationFunctionType.Exp

    nc = bass.Bass("TRN2", target_bir_lowering=False, debug=False,
                   enable_asserts=False)

    qT = nc.dram_tensor("qT", [C, T], bf16, kind="ExternalInput").ap()
    kT = nc.dram_tensor("kT", [C, T], bf16, kind="ExternalInput").ap()
    vT = nc.dram_tensor("vT", [C, T], bf16, kind="ExternalInput").ap()
    wq = nc.dram_tensor("wq", [C, HD], bf16, kind="ExternalInput").ap()
    wk = nc.dram_tensor("wk", [C, HD], bf16, kind="ExternalInput").ap()
    wv = nc.dram_tensor("wv", [C, HD], bf16, kind="ExternalInput").ap()
    wo = nc.dram_tensor("wo", [HD, C], f32, kind="ExternalInput").ap()
    out = nc.dram_tensor("out", [T, C], f32, kind="ExternalOutput").ap()

    with tile.TileContext(nc) as tc:
      for _rep in range(reps):
        with (
            tc.tile_pool(name="persist", bufs=1) as persist,
            tc.tile_pool(name="stage", bufs=3) as stage,
            tc.tile_pool(name="work", bufs=2) as work,
            tc.tile_pool(name="pt_pool", bufs=6) as pt_pool,
            tc.tile_pool(name="ps_proj", bufs=2, space="PSUM") as ps_proj,
            tc.tile_pool(name="ps_st", bufs=2, space="PSUM") as ps_st,
            tc.tile_pool(name="ps_ot", bufs=1, space="PSUM") as ps_ot,
        ):
            # ---- weights (loaded lazily, per phase; only wq gates start) ----
            wq_t = persist.tile([128, KC, HD], bf16)
            wk_t = persist.tile([128, KC, HD], bf16)
            wv_t = persist.tile([128, KC, HD], bf16)
            nc.sync.dma_start(wq_t[:], wq.rearrange("(o p) n -> p o n", p=128))

            # ---- projections ----
            # qhT/khT: [128 (pair-local head dim), NPAIR, T] f32r. Partition
            # p in pair m: head 2m for p<64, head 2m+1 for p>=64.
            qh_t = persist.tile([128, NPAIR, T], f32r)
            kh_t = persist.tile([128, NPAIR, T], f32r)
            # vh: [128 (T within tile), NT, HPC, 65] f32r; col 64 is ones
            # (softmax-denominator trick), cols 0..63 hold vh.
            # memset everything to 1; the projection copybacks overwrite
            # cols 0..63, leaving col 64 as the ones column.
            vh_t = persist.tile([128, NT, HPC, 65], bf16)
            nc.gpsimd.memset(vh_t[:, :, :, 64:65], 1.0)

            # causal mask tile: mask[p, c] = 1 if (c - 384) >= p else 0.
            # Diagonal block (i, j) with d = 128*i - 512*j uses the slice
            # mask[:, 384 - d : 896 - d]  (keep iff f - p >= d).
            mask_t = persist.tile([128, 896], bf16)
            nc.gpsimd.memset(mask_t[:], 1.0)
            nc.gpsimd.affine_select(
                mask_t[:], mask_t[:],
                compare_op=mybir.AluOpType.is_ge, fill=0.0,
                base=-384, pattern=[[1, 896]], channel_multiplier=-1,
            )

            # ---- pipelined: project half of T, then run its two
            # attention stripes while the other half projects ----
            def stage_quarter(src_ap, j):
                x_t = stage.tile([128, KC, 512], bf16, tag="x_stage")
                nc.gpsimd.dma_start(
                    x_t[:],
                    src_ap.rearrange("(o p) t -> p o t", p=128)[
                        :, :, j * 512:(j + 1) * 512],
                )
                return x_t

            def proj_qk(src_ap, w_t, dst, j):
                x_t = stage_quarter(src_ap, j)
                for m in range(NPAIR):
                    ps = ps_proj.tile([128, 512], f32, tag="proj")
                    for k in range(KC):
                        nc.tensor.matmul(
                            ps[:],
                            w_t[:, k, m * 128:(m + 1) * 128],
                            x_t[:, k, :],
                            start=(k == 0), stop=(k == KC - 1),
                        )
                    nc.scalar.copy(dst[:, m, j * 512:(j + 1) * 512], ps[:])

            def proj_v(j):
                x_t = stage_quarter(vT, j)
                for tt in range(4 * j, 4 * j + 4):
                    ps = ps_proj.tile([128, 512], f32, tag="proj")
                    off = tt * 128 - j * 512
                    for k in range(KC):
                        nc.tensor.matmul(
                            ps[:],
                            x_t[:, k, off:off + 128],
                            wv_t[:, k, :],
                            start=(k == 0), stop=(k == KC - 1),
                        )
                    nc.scalar.copy(
                        vh_t[:, tt, :, 0:64],
                        ps[:].rearrange("p (h d) -> p h d", h=HPC),
                    )

            def attention_stripe(j):
                nblk = 4 * j + 4
                for p in range(NPAIR):
                    ot0 = ps_ot.tile([128, 512], f32, tag="ot0")
                    ot1 = ps_ot.tile([128, 512], f32, tag="ot1")
                    for i in range(nblk):
                        # diagonal blocks (i >= 4j): columns [0, d) are
                        # entirely above the causal boundary -> skip them in
                        # the ST/PV matmuls and (when large) in the exp.
                        d = max(0, i * 128 - j * 512)
                        w = 512 - d
                        st = ps_st.tile([128, 1024], f32, tag="st")
                        # S^T block for both heads (row-group packed)
                        nc.tensor.matmul(
                            st[:, d:512],
                            kh_t[0:64, p, i * 128:(i + 1) * 128],
                            qh_t[0:64, p, j * 512 + d:(j + 1) * 512],
                            start=True, stop=True,
                        )
                        nc.tensor.matmul(
                            st[:, 512 + d:1024],
                            kh_t[64:128, p, i * 128:(i + 1) * 128],
                            qh_t[64:128, p, j * 512 + d:(j + 1) * 512],
                            start=True, stop=True,
                        )
                        pt = pt_pool.tile([128, 1024], bf16, tag="pt")
                        if d >= 256:
                            nc.scalar.activation(
                                pt[:, d:512], st[:, d:512], Exp, scale=SCALE)
                            nc.scalar.activation(
                                pt[:, 512 + d:1024], st[:, 512 + d:1024],
                                Exp, scale=SCALE)
                        else:
                            nc.scalar.activation(pt[:], st[:], Exp, scale=SCALE)
                        if i >= 4 * j:  # diagonal block: zero s > t entries
                            ms = mask_t[:, None, 384:896 - d]
                            nc.vector.tensor_tensor(
                                pt[:].rearrange("p (h f) -> p h f", h=2)[:, :, d:],
                                pt[:].rearrange("p (h f) -> p h f", h=2)[:, :, d:],
                                ms.to_broadcast((128, 2, w)),
                                mybir.AluOpType.mult,
                            )
                        # PV: accumulate [O^T; l] over key blocks
                        nc.tensor.matmul(
                            ot0[0:65, d:512],
                            vh_t[:, i, 2 * p, 0:65],
                            pt[:, d:512],
                            start=(i == 0), stop=(i == nblk - 1),
                        )
                        nc.tensor.matmul(
                            ot1[0:65, d:512],
                            vh_t[:, i, 2 * p + 1, 0:65],
                            pt[:, 512 + d:1024],
                            start=(i == 0), stop=(i == nblk - 1),
                        )
                    # epilogue: divide O^T rows by l. Copy O^T out of PSUM
                    # right away so the ot banks turn around fast; head 1
                    # additionally needs a partition shift 0..64 -> 64..128
                    # (small SBUF->SBUF DMA; lanes are partition-locked).
                    rc0 = work.tile([128, 512], f32, tag="recip")
                    rc1 = work.tile([128, 512], f32, tag="recip")
                    nc.vector.reciprocal(rc0[64:65, :], ot0[64:65, :])
                    nc.vector.reciprocal(rc1[64:65, :], ot1[64:65, :])
                    tmp0 = work.tile([128, 512], f32, tag="tmp0")
                    tmp1 = work.tile([128, 512], f32, tag="tmp1")
                    nc.vector.tensor_copy(tmp0[0:64, :], ot0[0:64, :])
                    nc.vector.tensor_copy(tmp1[0:64, :], ot1[0:64, :])
                    bc = work.tile([128, 512], f32, tag="bcast")
                    nc.sync.dma_start(
                        bc[0:64, :],
                        rc0[64:65, None, :].to_broadcast((1, 64, 512)),
                    )
                    nc.sync.dma_start(
                        bc[64:128, :],
                        rc1[64:65, None, :].to_broadcast((1, 64, 512)),
                    )
                    ash = work.tile([128, 512], f32, tag="ash")
                    nc.sync.dma_start(ash[64:128, :], tmp1[0:64, :])
                    nc.vector.tensor_tensor(
                        qh_t[0:64, p, j * 512:(j + 1) * 512],
                        tmp0[0:64, :], bc[0:64, :], mybir.AluOpType.mult,
                    )
                    nc.vector.tensor_tensor(
                        qh_t[64:128, p, j * 512:(j + 1) * 512],
                        ash[64:128, :], bc[64:128, :], mybir.AluOpType.mult,
                    )
            def outproj_stripe(j):
                # output projection for a stripe (A^T aliased into qh_t)
                for tt in range(4 * j, 4 * j + 4):
                    for cc in range(2):
                        ps = ps_proj.tile([128, 512], f32, tag="proj")
                        for kc in range(NPAIR):
                            nc.tensor.matmul(
                                ps[:],
                                qh_t[:, kc, tt * 128:(tt + 1) * 128],
                                wo_t[:, kc, cc * 512:(cc + 1) * 512],
                                start=(kc == 0), stop=(kc == NPAIR - 1),
                            )
                        ob = work.tile([128, 512], f32, tag="ob")
                        nc.vector.tensor_copy(ob[:], ps[:])
                        nc.sync.dma_start(
                            out[tt * 128:(tt + 1) * 128, cc * 512:(cc + 1) * 512],
                            ob[:],
                        )

            for half in range(2):
                js = (2 * half, 2 * half + 1)
                for j in js:
                    proj_qk(qT, wq_t, qh_t, j)
                if half == 0:
                    nc.sync.dma_start(
                        wk_t[:], wk.rearrange("(o p) n -> p o n", p=128))
                for j in js:
                    proj_qk(kT, wk_t, kh_t, j)
                if half == 0:
                    nc.sync.dma_start(
                        wv_t[:], wv.rearrange("(o p) n -> p o n", p=128))
                for j in js:
                    proj_v(j)
                if half == 0:
                    wo_f = stage.tile([128, HD // 128, C], f32, tag="wo_stage", bufs=1)
                    nc.sync.dma_start(
                        wo_f[:], wo.rearrange("(o p) n -> p o n", p=128))
                    wo_t_ = persist.tile([128, HD // 128, C], f32r, name="wo_t")
                    nc.gpsimd.tensor_copy(wo_t_[:], wo_f[:])
                    wo_t = wo_t_
                else:
                    outproj_stripe(0)
                    outproj_stripe(1)
                for j in js:
                    attention_stripe(j)
            outproj_stripe(2)
            outproj_stripe(3)

    _split_multi_waits(nc, mybir)
    return nc


def shard_inputs(k, q, v, Wk, Wq, Wv, Wo, bo):
    bf = ml_dtypes.bfloat16
    qT = [np.ascontiguousarray(q[b].T).astype(bf) for b in range(B)]
    kT = [np.ascontiguousarray(k[b].T).astype(bf) for b in range(B)]
    vT = [np.ascontiguousarray(v[b].T).astype(bf) for b in range(B)]

    def wslice(W, g):
        return np.ascontiguousarray(
            W[g * HPC:(g + 1) * HPC].transpose(1, 0, 2).reshape(C, HD)
        ).astype(bf)

    wq = [wslice(Wq, g) for g in range(2)]
    wk = [wslice(Wk, g) for g in range(2)]
    wv = [wslice(Wv, g) for g in range(2)]
    wo = [np.ascontiguousarray(Wo[g * HD:(g + 1) * HD]).astype(np.float32)
          for g in range(2)]
    in_maps = []
    for c in range(N_CORES):
        b, g = divmod(c, 2)
        in_maps.append({
            "qT": qT[b], "kT": kT[b], "vT": vT[b],
            "wq": wq[g], "wk": wk[g], "wv": wv[g], "wo": wo[g],
        })
    return in_maps


def gather_outputs(results, bo):
    out = np.empty((B, T, C), dtype=np.float32)
    for b in range(B):
        out[b] = results[2 * b]["out"] + results[2 * b + 1]["out"] + bo
    return out


def kernel(k, q, v, Wk, Wq, Wv, Wo, bo):
    from concourse.bass_utils import run_bass_kernel_spmd

    k = np.asarray(k, dtype=np.float32)
    q = np.asarray(q, dtype=np.float32)
    v = np.asarray(v, dtype=np.float32)
    Wk = np.asarray(Wk, dtype=np.float32)
    Wq = np.asarray(Wq, dtype=np.float32)
    Wv = np.asarray(Wv, dtype=np.float32)
    Wo = np.asarray(Wo, dtype=np.float32)
    bo = np.asarray(bo, dtype=np.float32)

    if "nc" not in _CACHE:
        _CACHE["nc"] = build_bass()
    nc = _CACHE["nc"]
    in_maps = shard_inputs(k, q, v, Wk, Wq, Wv, Wo, bo)
    res = run_bass_kernel_spmd(nc, in_maps, core_ids=list(range(N_CORES)))
    return gather_outputs(res.results, bo)
